# revision 28
# baseline (speedup 1.0000x reference)
"""Multi-head attention (B=16, N=577, C=768, H=12) on 8 TRN2 NeuronCores.

Strategy: pure data parallelism over batch (2 images per core, no
collectives). Per core, everything is computed "channels-on-partitions"
(transposed) so that no on-device transposes are ever needed:

  qkT[outc, tok]  = qkv_wT-tiles.T @ xT          (Wq pre-scaled 1/8 on host,
                                                  q bias added on DVE evict)
  V[tok, outc]    = xT-tiles.T @ qkv_wT          (natural layout; v_bias is
                                                  folded into the proj bias on
                                                  the host: pb += proj_w@v_bias)
  S^T[nk, nq]     = K^T-tiles.T @ Q^T            (K=64 contraction, row-tiled
                                                  pairs via base partitions)
  E^T             = exp(S^T) * exp(relbT)        (host precomputes exp of the
                                                  transposed rel-pos bias; no
                                                  max subtraction -- logits are
                                                  bounded ~|7| for this problem)
  O'^T[65, nq]    = [V_h | 1]-tiles.T @ E^T      (row 64 = softmax denominator)
  O^T             = O'^T[0:64] * bcast(1/O'^T[64])
  out^T[co, tok]  = projT-tiles.T @ O^T + proj_b

Performance structure (~185 us on silicon vs 224 us for the previous
baseline; engine profile: PE ~145 us active, DVE ~119, ACT ~107):
  - S chunks of BOTH heads of a pair land in one 2-bank PSUM tile; ONE
    exp instruction evicts both (halves the ACT instruction overhead);
    rumps accumulate in a shared tile with 2 strided 4D exp evicts
  - O' (mains [65,512] + rump [65,65]) of each head in ONE sp-tag PSUM
    tile -> single contiguous ost evict
  - QKV projection token-split across the two batch phases (PE stays
    dense in both phases so the HAM clock gate stays at full rate)
  - all inputs host-packed so every DMA is a contiguous 128-descriptor
    load; the QK weights load per block-pair so the first qk_groups
    start immediately; V(b0) tiles 2-4 are emitted mid-pair-0
  - output is written bf16 (halves out-DMA; host casts back to f32)
PSUM budget (8 banks exactly): big 2x[128,512] (dense), sp 2x[128,1024]
(S mains ping-pong + O' tiles + tail proj), rp 1x[128,1024] (S rumps).
"""
import numpy as np
import ml_dtypes

B, N, C, H, HD = 16, 577, 768, 12, 64
NCORES = 8
BPC = B // NCORES          # batches per core: 2
NT = BPC * N               # tokens per core: 1154
P = 128

# nk (key token) tiles over N
NKT = [(0, 128), (128, 128), (256, 128), (384, 128), (512, 65)]
# per-batch token chunks (free dim <= 512 for f32 psum)
PFREE = [
    [(0, 512), (512, 65)],           # batch 0 tokens
    [(577, 512), (1089, 65)],        # batch 1 tokens
]

_CACHE = {}


def _build():
    import concourse.tile as tile
    from concourse import bacc, mybir

    bf16 = mybir.dt.bfloat16
    f32 = mybir.dt.float32
    Alu = mybir.AluOpType
    Act = mybir.ActivationFunctionType

    nc = bacc.Bacc(
        "TRN2",
        target_bir_lowering=False,
        debug=False,
        enable_asserts=False,
        num_devices=NCORES,
    )
    # host-packed inputs: per-partition contiguous layouts
    xb = nc.dram_tensor("xb", [P, 2 * 6 * N], bf16, kind="ExternalInput").ap()
    wqk = nc.dram_tensor("wqk", [P, 6 * 2 * 6 * P], bf16, kind="ExternalInput").ap()
    wv = nc.dram_tensor("wv", [P, 6 * C], bf16, kind="ExternalInput").ap()
    qbias = nc.dram_tensor("qbias", [P, 6], f32, kind="ExternalInput").ap()
    relb = nc.dram_tensor("relb", [H, P, 5 * N], bf16, kind="ExternalInput").ap()
    projd = nc.dram_tensor("projd", [P, 6 * C], bf16, kind="ExternalInput").ap()
    pbias = nc.dram_tensor("pbias", [P, 6], f32, kind="ExternalInput").ap()
    out = nc.dram_tensor("out", [C, NT], bf16, kind="ExternalOutput").ap()

    with tile.TileContext(nc) as tc:
        with (
            tc.tile_pool(name="persist", bufs=1) as pp,
            tc.tile_pool(name="relb", bufs=2) as relp,
            tc.tile_pool(name="st", bufs=2) as stp,
            tc.tile_pool(name="dn", bufs=3) as dnp,
            tc.tile_pool(name="oev", bufs=3) as oevp,
            tc.tile_pool(name="psbig", bufs=2, space="PSUM") as ps_big,
            tc.tile_pool(name="pssp", bufs=2, space="PSUM") as ps_sp,
            tc.tile_pool(name="psrp", bufs=1, space="PSUM") as ps_rp,
        ):
            # ---------------- Phase A: load weights / constants ----------
            qb = pp.tile([P, 6], f32, tag="qb", name="qb")
            nc.sync.dma_start(qb[:], qbias[:])
            pb = pp.tile([P, 6], f32, tag="pb", name="pb")
            nc.sync.dma_start(pb[:], pbias[:])
            # x: [p, batch, ki, tok] -- batch 0 first
            xta = pp.tile([P, 2 * 6 * N], bf16, tag="xta", name="xta")
            xt = xta[:, :].rearrange("p (b k t) -> p b k t", b=2, k=6)
            xbv = xb[:, :].rearrange("p (b k t) -> p b k t", b=2, k=6)
            nc.sync.dma_start(xt[:, 0, :, :], xbv[:, 0, :, :])
            # QK weights: [p, blockpair(6), s(2:q/k), ki(6), c(128)] -- the
            # first block-pair loads alone so qk_group(0/6) start instantly
            wqa = pp.tile([P, 6 * 2 * 6 * P], bf16, tag="wqa", name="wqa")
            wq4 = wqa[:, :].rearrange("p (t s k c) -> p t s k c", t=6, s=2, k=6)
            wqv = wqk[:, :].rearrange("p (t s k c) -> p t s k c", t=6, s=2, k=6)
            nc.sync.dma_start(wq4[:, 0, :, :, :], wqv[:, 0, :, :, :])
            # V weights: [p, ki, c]
            wva = pp.tile([P, 6 * C], bf16, tag="wva", name="wva")
            wv3 = wva[:, :].rearrange("p (k c) -> p k c", k=6)
            nc.sync.dma_start(wva[:, :], wv[:, :])
            for bp in range(1, 6):
                nc.sync.dma_start(wq4[:, bp, :, :, :], wqv[:, bp, :, :, :])
            nc.sync.dma_start(xt[:, 1, :, :], xbv[:, 1, :, :])
            pta = pp.tile([P, 6 * C], bf16, tag="pta", name="pta")
            pt = pta[:, :].rearrange("p (k c) -> p k c", k=6)

            qk = []
            for t in range(12):
                qk.append(pp.tile([P, NT], bf16, tag=f"qk{t}", name=f"qk{t}"))
            o = []
            for t in range(6):
                o.append(pp.tile([P, NT], bf16, tag=f"o{t}", name=f"o{t}"))
            v = [[None] * 5 for _ in range(BPC)]

            def qk_group(t, half):
                # Q^T/K^T projection for outc block t, token half `half`
                s = 0 if t < 6 else 1
                bp = t % 6
                for (f0, fsz) in PFREE[half]:
                    ps = ps_big.tile([P, 512], f32, tag="big", name="psmm")
                    for ki in range(6):
                        nc.tensor.matmul(
                            ps[:, 0:fsz],
                            wq4[:, bp, s, ki, :],
                            xt[:, half, ki, f0 - half * N : f0 - half * N + fsz],
                            start=(ki == 0),
                            stop=(ki == 5),
                        )
                    if t < 6:  # q: bias add on DVE (Wq/q_bias pre-scaled on host)
                        nc.vector.tensor_scalar(
                            out=qk[t][:, f0 : f0 + fsz],
                            in0=ps[:, 0:fsz],
                            scalar1=qb[:, t : t + 1],
                            scalar2=None,
                            op0=Alu.add,
                        )
                    else:  # k: plain copy (k bias is zero)
                        nc.scalar.copy(qk[t][:, f0 : f0 + fsz], ps[:, 0:fsz])

            def v_group(b, j):
                # V projection (natural layout) for batch b, token tile j
                nk0, nksz = NKT[j]
                vt = pp.tile([P, 12 * 65], bf16, tag=f"v{b}_{j}", name=f"v{b}_{j}")
                v[b][j] = vt
                v3 = vt[:, :].rearrange("p (h w) -> p h w", w=65)
                nc.gpsimd.memset(v3[:, :, 64:65], 1.0)
                for half in range(2):  # outc halves of 384 = 6 heads
                    f0 = 384 * half
                    ps = ps_big.tile([P, 512], f32, tag="big", name="psmm")
                    for ki in range(6):
                        nc.tensor.matmul(
                            ps[0:nksz, 0:384],
                            xt[:, b, ki, nk0 : nk0 + nksz],
                            wv3[:, ki, f0 : f0 + 384],
                            start=(ki == 0),
                            stop=(ki == 5),
                        )
                    ps3 = ps[0:nksz, 0:384].rearrange("p (h w) -> p h w", w=64)
                    nc.vector.tensor_copy(
                        v3[0:nksz, 6 * half : 6 * half + 6, 0:64], ps3[:, :, :]
                    )

            pgcount = [0]

            def proj_group(t, half, pool=None):
                # both token chunks of (t, half), then one out-DMA
                ot = oevp.tile([P, N], bf16, tag="oev", name="oev")
                base = half * N
                for ci, (f0, fsz) in enumerate(PFREE[half]):
                    if pool is None or ci == 1:
                        ps = ps_big.tile([P, 512], f32, tag="big", name="psmm")
                    else:
                        ps = pool.tile([P, 1024], f32, tag="sp", name="psmm")
                    for ki in range(6):
                        nc.tensor.matmul(
                            ps[:, 0:fsz],
                            pt[:, ki, P * t : P * (t + 1)],
                            o[ki][:, f0 : f0 + fsz],
                            start=(ki == 0),
                            stop=(ki == 5),
                        )
                    pgcount[0] += 1
                    if pgcount[0] % 2 == 0:  # alternate ACT/DVE
                        nc.scalar.activation(
                            ot[:, f0 - base : f0 - base + fsz],
                            ps[:, 0:fsz],
                            Act.Identity,
                            bias=pb[:, t : t + 1],
                        )
                    else:
                        nc.vector.tensor_scalar(
                            out=ot[:, f0 - base : f0 - base + fsz],
                            in0=ps[:, 0:fsz],
                            scalar1=pb[:, t : t + 1],
                            scalar2=None,
                            op0=Alu.add,
                        )
                nc.sync.dma_start(
                    out[P * t : P * (t + 1), base : base + N], ot[:, 0:N]
                )

            def attention_pair(b, h0, mid_hook=None):
                qt = h0 // 2
                rba = {}
                for hh in (h0, h0 + 1):
                    pr = hh % 2
                    t = relp.tile([P, 5 * N], bf16, tag=f"rba{pr}", name=f"rba{pr}")
                    rba[hh] = t
                    nc.sync.dma_start(t[:, :], relb[hh, :, :])
                stam = stp.tile([P, 2 * 5 * N], bf16, tag="stam", name="stam")
                st4 = stam[:, :].rearrange("p (h j q) -> p h j q", j=5, q=N)
                rp = ps_rp.tile([P, 1024], f32, tag="rp", name="rps")
                rp4 = (
                    rp[:, :]
                    .rearrange("p (h x) -> p h x", h=2)[:, :, 0:325]
                    .rearrange("p h (j q) -> p h j q", q=65)
                )
                for j, (nk0, nksz) in enumerate(NKT):
                    sp = ps_sp.tile([P, 1024], f32, tag="sp", name="sps")
                    for idx, hh in enumerate((h0, h0 + 1)):
                        qoff = (hh % 2) * 64
                        lk = qk[6 + qt][
                            qoff : qoff + 64, b * N + nk0 : b * N + nk0 + nksz
                        ]
                        nc.tensor.matmul(
                            sp[0:nksz, 512 * idx : 512 * idx + 512],
                            lk,
                            qk[qt][qoff : qoff + 64, b * N : b * N + 512],
                            start=True,
                            stop=True,
                        )
                        nc.tensor.matmul(
                            rp[0:nksz, 512 * idx + 65 * j : 512 * idx + 65 * j + 65],
                            lk,
                            qk[qt][qoff : qoff + 64, b * N + 512 : b * N + N],
                            start=True,
                            stop=True,
                        )
                    nc.scalar.activation(
                        st4[:, :, j, 0:512],
                        sp[:, :].rearrange("p (h q) -> p h q", q=512),
                        Act.Exp,
                    )
                if mid_hook is not None:
                    mid_hook()
                nc.scalar.activation(
                    st4[:, :, 0:3, 512:577], rp4[:, :, 0:3, :], Act.Exp
                )
                nc.scalar.activation(
                    st4[:, :, 3:5, 512:577], rp4[:, :, 3:5, :], Act.Exp
                )
                for idx, hh in enumerate((h0, h0 + 1)):
                    sh = stam[:, idx * 5 * N : (idx + 1) * 5 * N]
                    nc.vector.tensor_tensor(
                        sh[:, 0 : 3 * N],
                        sh[:, 0 : 3 * N],
                        rba[hh][:, 0 : 3 * N],
                        op=Alu.mult,
                    )
                    nc.vector.tensor_tensor(
                        sh[:, 3 * N : 5 * N],
                        sh[:, 3 * N : 5 * N],
                        rba[hh][:, 3 * N : 5 * N],
                        op=Alu.mult,
                    )
                for idx, hh in enumerate((h0, h0 + 1)):
                    qoff = (hh % 2) * 64
                    om = ps_sp.tile([P, 1024], f32, tag="sp", name="om")
                    for j, (nk0, nksz) in enumerate(NKT):
                        lv = v[b][j][0:nksz, 65 * hh : 65 * hh + 65]
                        nc.tensor.matmul(
                            om[0:65, 0:512],
                            lv,
                            st4[0:nksz, idx : idx + 1, j : j + 1, 0:512],
                            start=(j == 0),
                            stop=(j == 4),
                        )
                        nc.tensor.matmul(
                            om[0:65, 512:577],
                            lv,
                            st4[0:nksz, idx : idx + 1, j : j + 1, 512:577],
                            start=(j == 0),
                            stop=(j == 4),
                        )
                    ost = dnp.tile([65, N], f32, tag="ost", name="ost")
                    nc.scalar.copy(ost[0:65, 0:N], om[0:65, 0:N])
                    dr = dnp.tile([1, N], f32, tag="dr", name="dr")
                    nc.vector.tensor_copy(dr[0:1, 0:N], ost[64:65, 0:N])
                    rr = dnp.tile([1, N], f32, tag="rr", name="rr")
                    nc.vector.reciprocal_approx_fast(rr[0:1, 0:N], dr[0:1, 0:N])
                    rb = dnp.tile([64, N], f32, tag="rbb", name="rbb")
                    nc.gpsimd.partition_broadcast(rb[0:64, 0:N], rr[0:1, 0:N])
                    nc.vector.tensor_tensor(
                        o[qt][qoff : qoff + 64, b * N : b * N + N],
                        ost[0:64, 0:N],
                        rb[0:64, 0:N],
                        op=Alu.mult,
                    )

            # ------------- interleaved emission schedule -------------------
            # qk(0)/qk(6) first (their weights load first) and STRAIGHT into
            # pair 0 -- all of V(b0) and the next pair's qk prefetch are
            # emitted mid-pair-0 (between its S and O' phases) so pair 0's
            # S matmuls are first in line after the qk evicts.
            qk_group(0, 0)
            qk_group(6, 0)

            def pair0_mid():
                for j in range(5):
                    v_group(0, j)
                qk_group(1, 0)
                qk_group(7, 0)

            for b in range(BPC):
                for h0 in range(0, 12, 2):
                    if h0 < 10 and (b, h0) != (0, 0):  # prefetch next pair's
                        qk_group(h0 // 2 + 1, b)       # QKV half
                        qk_group(7 + h0 // 2, b)
                    attention_pair(b, h0, mid_hook=pair0_mid if (b, h0) == (0, 0) else None)
                    if b == 0 and h0 == 4:
                        nc.sync.dma_start(pta[:, :], projd[:, :])
                    if b == 0 and h0 >= 6:
                        v_group(1, (h0 - 6) // 2 * 2)      # V(b1) late in b0
                        if (h0 - 6) // 2 * 2 + 1 < 5:
                            v_group(1, (h0 - 6) // 2 * 2 + 1)
                    if b == 0 and h0 == 10:  # prefetch first batch-1 qk halves
                        qk_group(0, 1)
                        qk_group(6, 1)
                    if b == 1:               # spread all batch-0 proj
                        proj_group(h0 // 2, 0)
            # ---------------- remaining output projection ------------------
            # draw psum from both pools so the first groups pre-accumulate
            # their first 5 k-tiles before the last head pair lands
            for t in range(6):
                proj_group(t, 1, pool=ps_sp if t % 2 == 0 else None)

    nc.compile()
    return nc


def _get_nc():
    if "nc" not in _CACHE:
        _CACHE["nc"] = _build()
    return _CACHE["nc"]


def make_in_maps(x, rel_pos_bias, qkv_w, q_bias, v_bias, proj_w, proj_b):
    bf = ml_dtypes.bfloat16
    x = np.asarray(x, dtype=np.float32)
    rel_pos_bias = np.asarray(rel_pos_bias, dtype=np.float32)
    qkv_w = np.asarray(qkv_w, dtype=np.float32)
    q_bias = np.asarray(q_bias, dtype=np.float32)
    v_bias = np.asarray(v_bias, dtype=np.float32)
    proj_w = np.asarray(proj_w, dtype=np.float32)
    proj_b = np.asarray(proj_b, dtype=np.float32)

    wqkv = qkv_w.copy()
    wqkv[0:C, :] *= 0.125                      # fold the 1/8 q scale into Wq
    wqkvT = np.ascontiguousarray(wqkv.T).astype(bf)                     # [768, 2304]
    # wqk packed: [p, blockpair(6), s(q/k), ki(6), c(128)]
    w5 = wqkvT[:, 0 : 2 * C].reshape(6, P, 2, 6, P)    # [ki, p, s, t, c]
    wqk = np.ascontiguousarray(w5.transpose(1, 3, 2, 0, 4).reshape(P, -1))
    # wv packed: [p, ki, c]
    wvp = np.ascontiguousarray(
        wqkvT[:, 2 * C : 3 * C].reshape(6, P, C).transpose(1, 0, 2).reshape(P, -1)
    )
    qbias = np.ascontiguousarray((q_bias * 0.125).reshape(6, P).T)      # [128, 6]
    # exp of the transposed rel-pos bias, packed [H, p, (j q)] so each
    # head's bias is one contiguous 128-descriptor DMA
    eb = np.exp(rel_pos_bias[0].transpose(0, 2, 1)).astype(bf)          # [H, nk, nq]
    relb = np.zeros((H, P, 5 * N), dtype=bf)
    for j in range(5):
        nk0, nksz = [(0, 128), (128, 128), (256, 128), (384, 128), (512, 65)][j]
        relb[:, 0:nksz, j * N : (j + 1) * N] = eb[:, nk0 : nk0 + nksz, :]
    projT = np.ascontiguousarray(proj_w.T).astype(bf)                   # [768, 768]
    projd = np.ascontiguousarray(
        projT.reshape(6, P, C).transpose(1, 0, 2).reshape(P, -1)
    )
    pbias_full = proj_b + proj_w @ v_bias
    pbias = np.ascontiguousarray(pbias_full.reshape(6, P).T)            # [128, 6]

    in_maps = []
    for c in range(NCORES):
        xTc = x[BPC * c : BPC * (c + 1)].reshape(2, N, C)               # [2, N, C]
        # xb packed: [p, b, ki, tok]
        xbp = np.ascontiguousarray(
            xTc.transpose(2, 0, 1).reshape(6, P, 2, N).transpose(1, 2, 0, 3).reshape(P, -1)
        ).astype(bf)
        in_maps.append(
            dict(
                xb=xbp,
                wqk=wqk,
                wv=wvp,
                qbias=qbias,
                relb=relb,
                projd=projd,
                pbias=pbias,
            )
        )
    return in_maps


def kernel(x, rel_pos_bias, qkv_w, q_bias, v_bias, proj_w, proj_b):
    from concourse import bass_utils

    in_maps = make_in_maps(x, rel_pos_bias, qkv_w, q_bias, v_bias, proj_w, proj_b)
    nc = _get_nc()
    res = bass_utils.run_bass_kernel_spmd(nc, in_maps, core_ids=list(range(NCORES)))
    outs = []
    for c in range(NCORES):
        oT = res.results[c]["out"]                                      # [768, 1154]
        outs.append(
            np.ascontiguousarray(oT.T).astype(np.float32).reshape(BPC, N, C)
        )
    return np.concatenate(outs, axis=0)


# revision 29
# speedup vs baseline: 1.0443x; 1.0443x over previous
"""Multi-head attention (B=16, N=577, C=768, H=12) on 8 TRN2 NeuronCores.

Strategy: pure data parallelism over batch (2 images per core, no
collectives). Per core, everything is computed "channels-on-partitions"
(transposed) so that no on-device transposes are ever needed:

  qkT[outc, tok]  = qkv_wT-tiles.T @ xT          (Wq pre-scaled 1/8 on host,
                                                  q bias added on DVE evict)
  V[tok, outc]    = xT-tiles.T @ qkv_wT          (natural layout; v_bias is
                                                  folded into the proj bias on
                                                  the host: pb += proj_w@v_bias)
  S^T[nk, nq]     = K^T-tiles.T @ Q^T            (K=64 contraction, row-tiled
                                                  pairs via base partitions)
  E^T             = exp(S^T) * exp(relbT)        (host precomputes exp of the
                                                  transposed rel-pos bias; no
                                                  max subtraction -- logits are
                                                  bounded ~|7| for this problem)
  O'^T[65, nq]    = [V_h | 1]-tiles.T @ E^T      (row 64 = softmax denominator)
  O^T             = O'^T[0:64] * bcast(1/O'^T[64])
  out^T[co, tok]  = projT-tiles.T @ O^T + proj_b

Performance structure (~185 us on silicon vs 224 us for the previous
baseline; engine profile: PE ~145 us active, DVE ~119, ACT ~107):
  - S chunks of BOTH heads of a pair land in one 2-bank PSUM tile; ONE
    exp instruction evicts both (halves the ACT instruction overhead);
    rumps accumulate in a shared tile with 2 strided 4D exp evicts
  - O' (mains [65,512] + rump [65,65]) of each head in ONE sp-tag PSUM
    tile -> single contiguous ost evict
  - QKV projection token-split across the two batch phases (PE stays
    dense in both phases so the HAM clock gate stays at full rate)
  - all inputs host-packed so every DMA is a contiguous 128-descriptor
    load; the QK weights load per block-pair so the first qk_groups
    start immediately; V(b0) tiles 2-4 are emitted mid-pair-0
  - output is written bf16 (halves out-DMA; host casts back to f32)
PSUM budget (8 banks exactly): big 2x[128,512] (dense), sp 2x[128,1024]
(S mains ping-pong + O' tiles + tail proj), rp 1x[128,1024] (S rumps).
"""
import numpy as np
import ml_dtypes

B, N, C, H, HD = 16, 577, 768, 12, 64
NCORES = 8
BPC = B // NCORES          # batches per core: 2
NT = BPC * N               # tokens per core: 1154
P = 128

# nk (key token) tiles over N
NKT = [(0, 128), (128, 128), (256, 128), (384, 128), (512, 65)]
# per-batch token chunks (free dim <= 512 for f32 psum)
PFREE = [
    [(0, 512), (512, 65)],           # batch 0 tokens
    [(577, 512), (1089, 65)],        # batch 1 tokens
]

_CACHE = {}


def _build():
    import concourse.tile as tile
    from concourse import bacc, mybir

    bf16 = mybir.dt.bfloat16
    f32 = mybir.dt.float32
    Alu = mybir.AluOpType
    Act = mybir.ActivationFunctionType

    nc = bacc.Bacc(
        "TRN2",
        target_bir_lowering=False,
        debug=False,
        enable_asserts=False,
        num_devices=NCORES,
    )
    # host-packed inputs: per-partition contiguous layouts
    xb = nc.dram_tensor("xb", [P, 2 * 6 * N], bf16, kind="ExternalInput").ap()
    wqk = nc.dram_tensor("wqk", [P, 6 * 2 * 6 * P], bf16, kind="ExternalInput").ap()
    wv = nc.dram_tensor("wv", [P, 6 * C], bf16, kind="ExternalInput").ap()
    qbias = nc.dram_tensor("qbias", [P, 6], f32, kind="ExternalInput").ap()
    relb = nc.dram_tensor("relb", [H, P, 5 * N], bf16, kind="ExternalInput").ap()
    projd = nc.dram_tensor("projd", [P, 6 * C], bf16, kind="ExternalInput").ap()
    pbias = nc.dram_tensor("pbias", [P, 6], f32, kind="ExternalInput").ap()
    out = nc.dram_tensor("out", [C, NT], bf16, kind="ExternalOutput").ap()

    with tile.TileContext(nc) as tc:
        with (
            tc.tile_pool(name="persist", bufs=1) as pp,
            tc.tile_pool(name="relb", bufs=2) as relp,
            tc.tile_pool(name="st", bufs=2) as stp,
            tc.tile_pool(name="dn", bufs=3) as dnp,
            tc.tile_pool(name="oev", bufs=3) as oevp,
            tc.tile_pool(name="psbig", bufs=2, space="PSUM") as ps_big,
            tc.tile_pool(name="pssp", bufs=2, space="PSUM") as ps_sp,
            tc.tile_pool(name="psrp", bufs=1, space="PSUM") as ps_rp,
        ):
            # ---------------- Phase A: load weights / constants ----------
            qb = pp.tile([P, 6], f32, tag="qb", name="qb")
            nc.sync.dma_start(qb[:], qbias[:])
            pb = pp.tile([P, 6], f32, tag="pb", name="pb")
            nc.sync.dma_start(pb[:], pbias[:])
            # x: [p, batch, ki, tok] -- batch 0 first
            xta = pp.tile([P, 2 * 6 * N], bf16, tag="xta", name="xta")
            xt = xta[:, :].rearrange("p (b k t) -> p b k t", b=2, k=6)
            xbv = xb[:, :].rearrange("p (b k t) -> p b k t", b=2, k=6)
            nc.sync.dma_start(xt[:, 0, :, :], xbv[:, 0, :, :])
            # QK weights: [p, blockpair(6), s(2:q/k), ki(6), c(128)] -- the
            # first block-pair loads alone so qk_group(0/6) start instantly
            wqa = pp.tile([P, 6 * 2 * 6 * P], bf16, tag="wqa", name="wqa")
            wq4 = wqa[:, :].rearrange("p (t s k c) -> p t s k c", t=6, s=2, k=6)
            wqv = wqk[:, :].rearrange("p (t s k c) -> p t s k c", t=6, s=2, k=6)
            nc.sync.dma_start(wq4[:, 0, :, :, :], wqv[:, 0, :, :, :])
            # V weights: [p, ki, c]
            wva = pp.tile([P, 6 * C], bf16, tag="wva", name="wva")
            wv3 = wva[:, :].rearrange("p (k c) -> p k c", k=6)
            nc.sync.dma_start(wva[:, :], wv[:, :])
            for bp in range(1, 6):
                nc.sync.dma_start(wq4[:, bp, :, :, :], wqv[:, bp, :, :, :])
            nc.sync.dma_start(xt[:, 1, :, :], xbv[:, 1, :, :])
            pta = pp.tile([P, 6 * C], bf16, tag="pta", name="pta")
            pt = pta[:, :].rearrange("p (k c) -> p k c", k=6)

            qk = []
            for t in range(12):
                qk.append(pp.tile([P, NT], bf16, tag=f"qk{t}", name=f"qk{t}"))
            o = []
            for t in range(6):
                o.append(pp.tile([P, NT], bf16, tag=f"o{t}", name=f"o{t}"))
            v = [[None] * 5 for _ in range(BPC)]

            def qk_group(t, half):
                # Q^T/K^T projection for outc block t, token half `half`
                s = 0 if t < 6 else 1
                bp = t % 6
                for (f0, fsz) in PFREE[half]:
                    ps = ps_big.tile([P, 512], f32, tag="big", name="psmm")
                    for ki in range(6):
                        nc.tensor.matmul(
                            ps[:, 0:fsz],
                            wq4[:, bp, s, ki, :],
                            xt[:, half, ki, f0 - half * N : f0 - half * N + fsz],
                            start=(ki == 0),
                            stop=(ki == 5),
                        )
                    if t < 6:  # q: bias add on DVE (Wq/q_bias pre-scaled on host)
                        nc.vector.tensor_scalar(
                            out=qk[t][:, f0 : f0 + fsz],
                            in0=ps[:, 0:fsz],
                            scalar1=qb[:, t : t + 1],
                            scalar2=None,
                            op0=Alu.add,
                        )
                    else:  # k: plain copy (k bias is zero)
                        nc.scalar.copy(qk[t][:, f0 : f0 + fsz], ps[:, 0:fsz])

            def v_group(b, j):
                # V projection (natural layout) for batch b, token tile j
                nk0, nksz = NKT[j]
                vt = pp.tile([P, 12 * 65], bf16, tag=f"v{b}_{j}", name=f"v{b}_{j}")
                v[b][j] = vt
                v3 = vt[:, :].rearrange("p (h w) -> p h w", w=65)
                nc.gpsimd.memset(v3[:, :, 64:65], 1.0)
                for half in range(2):  # outc halves of 384 = 6 heads
                    f0 = 384 * half
                    ps = ps_big.tile([P, 512], f32, tag="big", name="psmm")
                    for ki in range(6):
                        nc.tensor.matmul(
                            ps[0:nksz, 0:384],
                            xt[:, b, ki, nk0 : nk0 + nksz],
                            wv3[:, ki, f0 : f0 + 384],
                            start=(ki == 0),
                            stop=(ki == 5),
                        )
                    ps3 = ps[0:nksz, 0:384].rearrange("p (h w) -> p h w", w=64)
                    nc.vector.tensor_copy(
                        v3[0:nksz, 6 * half : 6 * half + 6, 0:64], ps3[:, :, :]
                    )

            pgcount = [0]

            def proj_group(t, half, pool=None):
                # both token chunks of (t, half), then one out-DMA
                ot = oevp.tile([P, N], bf16, tag="oev", name="oev")
                base = half * N
                for ci, (f0, fsz) in enumerate(PFREE[half]):
                    if pool is None or ci == 1:
                        ps = ps_big.tile([P, 512], f32, tag="big", name="psmm")
                    else:
                        ps = pool.tile([P, 1024], f32, tag="sp", name="psmm")
                    for ki in range(6):
                        nc.tensor.matmul(
                            ps[:, 0:fsz],
                            pt[:, ki, P * t : P * (t + 1)],
                            o[ki][:, f0 : f0 + fsz],
                            start=(ki == 0),
                            stop=(ki == 5),
                        )
                    pgcount[0] += 1
                    if pgcount[0] % 2 == 0:  # alternate ACT/DVE
                        nc.scalar.activation(
                            ot[:, f0 - base : f0 - base + fsz],
                            ps[:, 0:fsz],
                            Act.Identity,
                            bias=pb[:, t : t + 1],
                        )
                    else:
                        nc.vector.tensor_scalar(
                            out=ot[:, f0 - base : f0 - base + fsz],
                            in0=ps[:, 0:fsz],
                            scalar1=pb[:, t : t + 1],
                            scalar2=None,
                            op0=Alu.add,
                        )
                nc.sync.dma_start(
                    out[P * t : P * (t + 1), base : base + N], ot[:, 0:N]
                )

            def attention_pair(b, h0, mid_hook=None):
                qt = h0 // 2
                rba = {}
                for hh in (h0, h0 + 1):
                    pr = hh % 2
                    t = relp.tile([P, 5 * N], bf16, tag=f"rba{pr}", name=f"rba{pr}")
                    rba[hh] = t
                    nc.sync.dma_start(t[:, :], relb[hh, :, :])
                stam = stp.tile([P, 2 * 5 * N], bf16, tag="stam", name="stam")
                st4 = stam[:, :].rearrange("p (h j q) -> p h j q", j=5, q=N)
                rp = ps_rp.tile([P, 1024], f32, tag="rp", name="rps")
                rp4 = (
                    rp[:, :]
                    .rearrange("p (h x) -> p h x", h=2)[:, :, 0:325]
                    .rearrange("p h (j q) -> p h j q", q=65)
                )
                for j, (nk0, nksz) in enumerate(NKT):
                    sp = ps_sp.tile([P, 1024], f32, tag="sp", name="sps")
                    for idx, hh in enumerate((h0, h0 + 1)):
                        qoff = (hh % 2) * 64
                        lk = qk[6 + qt][
                            qoff : qoff + 64, b * N + nk0 : b * N + nk0 + nksz
                        ]
                        nc.tensor.matmul(
                            sp[0:nksz, 512 * idx : 512 * idx + 512],
                            lk,
                            qk[qt][qoff : qoff + 64, b * N : b * N + 512],
                            start=True,
                            stop=True,
                        )
                        nc.tensor.matmul(
                            rp[0:nksz, 512 * idx + 65 * j : 512 * idx + 65 * j + 65],
                            lk,
                            qk[qt][qoff : qoff + 64, b * N + 512 : b * N + N],
                            start=True,
                            stop=True,
                        )
                    nc.scalar.activation(
                        st4[:, :, j, 0:512],
                        sp[:, :].rearrange("p (h q) -> p h q", q=512),
                        Act.Exp,
                    )
                if mid_hook is not None:
                    mid_hook()
                nc.scalar.activation(
                    st4[:, :, 0:3, 512:577], rp4[:, :, 0:3, :], Act.Exp
                )
                nc.scalar.activation(
                    st4[:, :, 3:5, 512:577], rp4[:, :, 3:5, :], Act.Exp
                )
                for idx, hh in enumerate((h0, h0 + 1)):
                    sh = stam[:, idx * 5 * N : (idx + 1) * 5 * N]
                    nc.vector.tensor_tensor(
                        sh[:, 0 : 3 * N],
                        sh[:, 0 : 3 * N],
                        rba[hh][:, 0 : 3 * N],
                        op=Alu.mult,
                    )
                    nc.vector.tensor_tensor(
                        sh[:, 3 * N : 5 * N],
                        sh[:, 3 * N : 5 * N],
                        rba[hh][:, 3 * N : 5 * N],
                        op=Alu.mult,
                    )
                for idx, hh in enumerate((h0, h0 + 1)):
                    qoff = (hh % 2) * 64
                    om = ps_sp.tile([P, 1024], f32, tag="sp", name="om")
                    for j, (nk0, nksz) in enumerate(NKT):
                        lv = v[b][j][0:nksz, 65 * hh : 65 * hh + 65]
                        nc.tensor.matmul(
                            om[0:65, 0:512],
                            lv,
                            st4[0:nksz, idx : idx + 1, j : j + 1, 0:512],
                            start=(j == 0),
                            stop=(j == 4),
                        )
                        nc.tensor.matmul(
                            om[0:65, 512:577],
                            lv,
                            st4[0:nksz, idx : idx + 1, j : j + 1, 512:577],
                            start=(j == 0),
                            stop=(j == 4),
                        )
                    ost = dnp.tile([65, N], f32, tag="ost", name="ost")
                    nc.scalar.copy(ost[0:65, 0:N], om[0:65, 0:N])
                    dr = dnp.tile([1, N], f32, tag="dr", name="dr")
                    nc.vector.tensor_copy(dr[0:1, 0:N], ost[64:65, 0:N])
                    rr = dnp.tile([1, N], f32, tag="rr", name="rr")
                    nc.vector.reciprocal_approx_fast(rr[0:1, 0:N], dr[0:1, 0:N])
                    rb = dnp.tile([64, N], f32, tag="rbb", name="rbb")
                    nc.gpsimd.partition_broadcast(rb[0:64, 0:N], rr[0:1, 0:N])
                    nc.vector.tensor_tensor(
                        o[qt][qoff : qoff + 64, b * N : b * N + N],
                        ost[0:64, 0:N],
                        rb[0:64, 0:N],
                        op=Alu.mult,
                    )

            # ------------- interleaved emission schedule -------------------
            # qk(0)/qk(6) first (their weights load first), then 2 v_groups;
            # V(b0) tiles 2-4 are emitted mid-pair-0 (between S and O').
            qk_group(0, 0)
            qk_group(6, 0)
            v_group(0, 0)
            v_group(0, 1)

            def pair0_mid():
                for j in range(2, 5):
                    v_group(0, j)

            for b in range(BPC):
                for h0 in range(0, 12, 2):
                    if h0 < 10:              # prefetch next pair's QKV half
                        qk_group(h0 // 2 + 1, b)
                        qk_group(7 + h0 // 2, b)
                    attention_pair(b, h0, mid_hook=pair0_mid if (b, h0) == (0, 0) else None)
                    if b == 0 and h0 == 4:
                        nc.sync.dma_start(pta[:, :], projd[:, :])
                    if b == 0 and h0 >= 6:
                        v_group(1, (h0 - 6) // 2 * 2)      # V(b1) late in b0
                        if (h0 - 6) // 2 * 2 + 1 < 5:
                            v_group(1, (h0 - 6) // 2 * 2 + 1)
                    if b == 0 and h0 == 10:  # prefetch first batch-1 qk halves
                        qk_group(0, 1)
                        qk_group(6, 1)
                    if b == 1:               # spread all batch-0 proj
                        proj_group(h0 // 2, 0)
            # ---------------- remaining output projection ------------------
            # draw psum from both pools so the first groups pre-accumulate
            # their first 5 k-tiles before the last head pair lands
            for t in range(6):
                proj_group(t, 1, pool=ps_sp if t % 2 == 0 else None)

    nc.compile()
    return nc


def _get_nc():
    if "nc" not in _CACHE:
        _CACHE["nc"] = _build()
    return _CACHE["nc"]


def make_in_maps(x, rel_pos_bias, qkv_w, q_bias, v_bias, proj_w, proj_b):
    bf = ml_dtypes.bfloat16
    x = np.asarray(x, dtype=np.float32)
    rel_pos_bias = np.asarray(rel_pos_bias, dtype=np.float32)
    qkv_w = np.asarray(qkv_w, dtype=np.float32)
    q_bias = np.asarray(q_bias, dtype=np.float32)
    v_bias = np.asarray(v_bias, dtype=np.float32)
    proj_w = np.asarray(proj_w, dtype=np.float32)
    proj_b = np.asarray(proj_b, dtype=np.float32)

    wqkv = qkv_w.copy()
    wqkv[0:C, :] *= 0.125                      # fold the 1/8 q scale into Wq
    wqkvT = np.ascontiguousarray(wqkv.T).astype(bf)                     # [768, 2304]
    # wqk packed: [p, blockpair(6), s(q/k), ki(6), c(128)]
    w5 = wqkvT[:, 0 : 2 * C].reshape(6, P, 2, 6, P)    # [ki, p, s, t, c]
    wqk = np.ascontiguousarray(w5.transpose(1, 3, 2, 0, 4).reshape(P, -1))
    # wv packed: [p, ki, c]
    wvp = np.ascontiguousarray(
        wqkvT[:, 2 * C : 3 * C].reshape(6, P, C).transpose(1, 0, 2).reshape(P, -1)
    )
    qbias = np.ascontiguousarray((q_bias * 0.125).reshape(6, P).T)      # [128, 6]
    # exp of the transposed rel-pos bias, packed [H, p, (j q)] so each
    # head's bias is one contiguous 128-descriptor DMA
    eb = np.exp(rel_pos_bias[0].transpose(0, 2, 1)).astype(bf)          # [H, nk, nq]
    relb = np.zeros((H, P, 5 * N), dtype=bf)
    for j in range(5):
        nk0, nksz = [(0, 128), (128, 128), (256, 128), (384, 128), (512, 65)][j]
        relb[:, 0:nksz, j * N : (j + 1) * N] = eb[:, nk0 : nk0 + nksz, :]
    projT = np.ascontiguousarray(proj_w.T).astype(bf)                   # [768, 768]
    projd = np.ascontiguousarray(
        projT.reshape(6, P, C).transpose(1, 0, 2).reshape(P, -1)
    )
    pbias_full = proj_b + proj_w @ v_bias
    pbias = np.ascontiguousarray(pbias_full.reshape(6, P).T)            # [128, 6]

    in_maps = []
    for c in range(NCORES):
        xTc = x[BPC * c : BPC * (c + 1)].reshape(2, N, C)               # [2, N, C]
        # xb packed: [p, b, ki, tok]
        xbp = np.ascontiguousarray(
            xTc.transpose(2, 0, 1).reshape(6, P, 2, N).transpose(1, 2, 0, 3).reshape(P, -1)
        ).astype(bf)
        in_maps.append(
            dict(
                xb=xbp,
                wqk=wqk,
                wv=wvp,
                qbias=qbias,
                relb=relb,
                projd=projd,
                pbias=pbias,
            )
        )
    return in_maps


def kernel(x, rel_pos_bias, qkv_w, q_bias, v_bias, proj_w, proj_b):
    from concourse import bass_utils

    in_maps = make_in_maps(x, rel_pos_bias, qkv_w, q_bias, v_bias, proj_w, proj_b)
    nc = _get_nc()
    res = bass_utils.run_bass_kernel_spmd(nc, in_maps, core_ids=list(range(NCORES)))
    outs = []
    for c in range(NCORES):
        oT = res.results[c]["out"]                                      # [768, 1154]
        outs.append(
            np.ascontiguousarray(oT.T).astype(np.float32).reshape(BPC, N, C)
        )
    return np.concatenate(outs, axis=0)
